# revision 61
# baseline (speedup 1.0000x reference)
"""Trainium2 Bass kernel for nn_FISLayerParameterSharingV2.

Math: dcumsum along an axis with discount d is multiplication by a lower
triangular matrix L[i,j] = d^(i-j).  With H = W = 128 the whole per-(b,t)
chain is expressible as 128x128 matmuls + elementwise products:

    s3  = Ls Z3 Ls^T          (Ls strict lower triangular)
    s2  = Ls (Z2*s3) Ls^T
    out = L  (Z1*s2) L^T      (L inclusive lower triangular)

Kernel layout strategy (per core; B is sharded 4 per core over 8 cores):

  *  "einsum-F": matmul(lhsT=x[b,:,h,:] (c,w), rhs=alphaT (c,3T)) emits
     Z^T tiles [w, 3T] per (b,h) -- channel contraction AND the t->pixel
     layout pivot in a single PE op.  Outputs are packed in PSUM and
     evacuated into a per-b SBUF buffer Bp[w, (k,t,h)] (fp16).
  *  Each stage runs in transposed space [w, h]: the left Ls-multiply is a
     PE matmul (contraction over w), the right Ls^T multiply is a
     discounted inclusive scan along free h on VectorE
     (tensor_tensor_scan, multiplier tensor with 0 at h=0 per t-block to
     reset), which also performs the PSUM->SBUF evacuation.  The
     strict-shift is an AP offset into the scan result + a column memset;
     the leftover d factors fold into alpha_1's scaling.
  *  Fast path (uniform discount, the common case): stationaries/masks are
     shared across t, so each stage is ONE 512-wide matmul per 4-t group
     (shared lhsT) instead of four 128-wide ones, and the final stage's
     un-transposing matmul is operand-swapped (lhsT=frhs stationary,
     rhs=n1) emitting [w', (t,h)]; output is stored f16 in [b, w, t, h]
     DRAM layout (2KB contiguous rows) and fixed up on host
     (transpose + 2^20 unscale).  The schedule is software-pipelined over
     32 (b, 4-t-group) units with every DVE op's producer issued a tick
     early; einsum pieces for later b's are paced into the stage ticks;
     late units do the final h-cumsum as a second PE matmul instead of a
     DVE scan (DVE paces the tail, Act idles there).  Engine splits are
     set by the swept _TUNE knobs.
  *  General path (per-t discounts): original per-t matmul variant with
     [128, T*128] stationaries kept as a fallback.
  *  fp16 storage everywhere with power-of-2 prescales folded into the
     alphas (the scans amplify ~50x per stage and would overflow fp16).
"""

import sys
import numpy as np

for _p in ("/opt/trn_rl_repo",):
    if _p not in sys.path:
        sys.path.insert(0, _p)

B, T, C, H, W = 32, 32, 64, 128, 128
NCORES = 8
BPC = B // NCORES          # batches per core
NG_HOST = 8                # t-quad groups per b (mirrors NG in the builder)
KA = 3                     # number of alphas
C1, C2, C3 = 2.0 ** -8, 2.0 ** -6, 2.0 ** -6
UNSCALE = 1.0 / (C1 * C2 * C3)

_CACHE = {}
_LAST_KEY = None

# schedule tuning knobs (resolved by sweep; see _build_module_uniform)
_TUNE = {
    "b1_head": 0,        # how many of b1's einsum pieces run in the head
    "extra_rate": 4,     # einsum pieces injected per stage tick
    "extra_dve_mod": 6,  # every Nth extra evac on DVE (0 = none)
    "stg_dve_mod": 0,    # every Nth stg evac on DVE (0 = none)
    "mul2_dve_mod": 3,   # every Nth m2 mul on DVE (0 = none)
    "mul1_dve_mod": 3,   # every Nth m1 mul on DVE (0 = none)
    "consts_queue": "sync",
    # units >= this index do the final h-cumsum as a second PE matmul
    # (double-MM final; output layout [h,t,w] instead of [w,t,h]) rather
    # than a DVE scan -- Act idles late while DVE paces, so trading a
    # scan for an extra evac there wins.  Must be a multiple of 2.
    "fin_mm_from": 16,
    "fin_mm_to": 30,     # window end: units >= this revert to scan-final
    "qa_dve_mod": 0,     # every Nth double-MM mid-evac on DVE (0 = none)
    # dummy warm-up matmuls issued before the first x chunk arrives, to
    # ramp the PE out of its low p-state before the real einsum begins
    "warm_mms": 0,
    "extra_start": 0,    # first stage tick that injects einsum extras
    "head_act_mod": 3,   # b0 evacs: every Nth on DVE, rest Act
    "split_dma_from": 24,  # units >= this DMA per 4-t group (must be even)
    "yp_bufs": 6, "mp_bufs": 6, "st_bufs": 4, "n1_bufs": 4,
}


def _build_module_uniform():
    """Fast path: one discount shared by all T trees."""
    import concourse.bass as bass
    import concourse.mybir as mybir
    import concourse.tile as tile
    from concourse import bacc
    from contextlib import ExitStack

    dt = mybir.dt
    f32, f16 = dt.float32, dt.float16

    nc = bacc.Bacc(
        "TRN2", target_bir_lowering=False, debug=False, num_devices=NCORES
    )
    # x arrives pre-cast to f16 (DMA cost is dst-bytes either way, but a
    # non-casting DMA can be issued from any queue, not just gpsimd)
    xs = nc.declare_dram_parameter("xs", [BPC, C, H, W], f16, isOutput=False)
    alphaT = nc.declare_dram_parameter("alphaT", [128, KA * T], f16, isOutput=False)
    statT = nc.declare_dram_parameter("statT", [128, 128], f16, isOutput=False)
    frhsT = nc.declare_dram_parameter("frhsT", [128, 128], f16, isOutput=False)
    dmask = nc.declare_dram_parameter("dmask", [128, 512], f32, isOutput=False)
    # output transposed+scaled: outT[b, w, t, h] = out[b, t, h, w] * 2^-20
    outT = nc.declare_dram_parameter("outT", [BPC, W, T, H], f16, isOutput=True)

    HB = 32                    # h-block size for x streaming
    NHB = H // HB              # 4 h-blocks
    NG = T // 4                # 8 t-quad groups
    MULT = mybir.AluOpType.mult
    ADD = mybir.AluOpType.add

    with tile.TileContext(nc) as tc, ExitStack() as ctx:
        const_pool = ctx.enter_context(tc.tile_pool(name="const", bufs=1))
        xpool = ctx.enter_context(tc.tile_pool(name="xp", bufs=8))
        bppool = ctx.enter_context(tc.tile_pool(name="bp", bufs=4))
        ypool = ctx.enter_context(tc.tile_pool(name="yp", bufs=_TUNE["yp_bufs"]))
        mpool = ctx.enter_context(tc.tile_pool(name="mp", bufs=_TUNE["mp_bufs"]))
        n1pool = ctx.enter_context(tc.tile_pool(name="n1", bufs=_TUNE["n1_bufs"]))
        stpool = ctx.enter_context(tc.tile_pool(name="st", bufs=_TUNE["st_bufs"]))
        pspool = ctx.enter_context(tc.tile_pool(name="ps", bufs=8, space="PSUM"))

        # consts go through the gpsimd SWDGE queue: a SWDGE issue costs ~1us
        # of Pool engine (idle here) and does NOT hold its sequencer through
        # the transfer, unlike HWDGE queues which would serialize ~2.5us per
        # DMA ahead of the latency-critical x stream.
        alpha_t = const_pool.tile([128, KA * T], f16, tag="alpha")
        nc.sync.dma_start(alpha_t[:], alphaT[:])
        st_t = const_pool.tile([128, 128], f16, tag="stm")
        nc.sync.dma_start(st_t[:], statT[:])
        fr_t = const_pool.tile([128, 128], f16, tag="frm")
        nc.sync.dma_start(fr_t[:], frhsT[:])
        dm_t = const_pool.tile([128, 512], f32, tag="dmm")
        nc.sync.dma_start(dm_t[:], dmask[:])

        bp_tiles = {}   # b -> tile
        bpv = {}        # b -> rearranged view

        def make_bp(b):
            bp_tiles[b] = bppool.tile(
                [128, KA * T * 128], f16, tag="bp", name=f"bp{b}"
            )
            bpv[b] = bp_tiles[b][:].rearrange(
                "p (k t h) -> p k t h", k=KA, t=T
            )

        def zero_shift_cols(b):
            # m2/m1's strict-shift columns (h=0 per t) must come out zero;
            # zeroing z1/z2's h=0 columns once makes every later elementwise
            # product there zero, replacing the per-group memsets.  (The
            # product's other operand is uninitialized there, as in the
            # per-group variant, which also read it before zeroing.)
            nc.gpsimd.memset(bpv[b][:, 0:2, :, 0:1], 0.0)

        # x chunks: one SBUF tile per (pair, hb) shared by its two b's.
        # b0's halves go via SWDGE (fast issue; the head is latency-bound on
        # them), b1's via SP HWDGE (needed only by tick 8, and SP is free),
        # pair 1 as full chunks via SWDGE mid-schedule.
        x_tiles = {}

        def ensure_x(b, hb):
            pair, half = b // 2, b % 2
            if (pair, hb) not in x_tiles:
                x_tiles[(pair, hb)] = xpool.tile(
                    [128, HB * W], f16, tag="x", name=f"xt{pair}_{hb}"
                )
            xt = x_tiles[(pair, hb)]
            if pair == 0:
                if (pair, hb, half) not in x_tiles:
                    x_tiles[(pair, hb, half)] = True
                    src = xs[b, :, hb * HB : (hb + 1) * HB, :]
                    dma = nc.gpsimd if b == 0 else nc.sync
                    dma.dma_start(
                        xt[64 * half : 64 * half + 64, :],
                        src.rearrange("c h w -> c (h w)"),
                    )
            else:
                if (pair, hb, "full") not in x_tiles:
                    x_tiles[(pair, hb, "full")] = True
                    src = xs[2:4, :, hb * HB : (hb + 1) * HB, :]
                    nc.gpsimd.dma_start(
                        xt[:], src.rearrange("b c h w -> (b c) (h w)")
                    )
            return xt

        def einsum_pieces(b):
            """Generator of closures: x-DMA + MM-group + pivot-evac pieces."""
            for hb in range(NHB):
                joff = 0
                for ui, ng in enumerate((5, 5, 5, 5, 4, 4, 4)):
                    def piece(b=b, hb=hb, ng=ng, joff=joff, ui=ui):
                        xt = ensure_x(b, hb)
                        half = b % 2
                        pts = pspool.tile(
                            [128, ng * KA * T], f32, tag="ps", name="pe"
                        )
                        for j in range(ng):
                            nc.tensor.matmul(
                                pts[:, j * 96 : (j + 1) * 96],
                                lhsT=xt[
                                    64 * half : 64 * half + 64,
                                    (joff + j) * W : (joff + j + 1) * W,
                                ],
                                rhs=alpha_t[64 * half : 64 * half + 64, :],
                                tile_position=(64 * half, 0),
                                skip_group_check=True,
                            )
                        h0 = hb * HB + joff
                        src_ap = pts[:].rearrange(
                            "p (j k t) -> p j k t", j=ng, k=KA
                        )
                        dst_ap = (
                            bpv[b][:, :, :, h0 : h0 + ng]
                            .rearrange("p k t j -> p j k t")
                        )
                        gi = hb * 7 + ui
                        # head phase (b0): Act and DVE split evenly (gpsimd
                        # cannot read PSUM); steady phase: mostly Act
                        em = _TUNE["extra_dve_mod"]
                        hm = _TUNE["head_act_mod"]
                        use_dve = (b == 0 and gi % hm == 0) or (
                            b > 0 and em and gi % em == em - 1
                        )
                        if use_dve:
                            nc.vector.tensor_copy(dst_ap, src_ap)
                        else:
                            nc.scalar.copy(dst_ap, src_ap)
                    yield piece
                    joff += ng

        # per-(b, group) live state for the software-pipelined stages
        live = {}

        def s3mms(b, g):
            t0 = 4 * g
            p3 = pspool.tile([128, 512], f32, tag="ps", name="p3")
            nc.tensor.matmul(
                p3[:],
                lhsT=st_t[:],
                rhs=bpv[b][:, 2, t0 : t0 + 4, :].rearrange("p t h -> p (t h)"),
                skip_group_check=True,
            )
            live[(b, g, "p3")] = p3

        def scan3_mul2(b, g):
            t0 = 4 * g
            p3 = live.pop((b, g, "p3"))
            y3 = ypool.tile([128, 516], f16, tag="y", name="y3")
            nc.vector.tensor_tensor_scan(
                y3[:, 1:513], dm_t[:], p3[:],
                initial=0.0, op0=MULT, op1=ADD,
            )
            m2 = mpool.tile([128, 512], f16, tag="m", name="m2")
            # alternate per unit so a tick never lands both muls on Pool
            m2m = _TUNE["mul2_dve_mod"]
            eng = (nc.vector if m2m and (b * NG + g) % m2m == 0
                   else nc.gpsimd)
            eng.tensor_mul(
                m2[:],
                bpv[b][:, 1, t0 : t0 + 4, :].rearrange("p t h -> p (t h)"),
                y3[:, 0:512],
            )
            live[(b, g, "m2")] = m2

        def s2mms(b, g):
            m2 = live.pop((b, g, "m2"))
            p2 = pspool.tile([128, 512], f32, tag="ps", name="p2")
            nc.tensor.matmul(
                p2[:], lhsT=st_t[:], rhs=m2[:], skip_group_check=True,
            )
            live[(b, g, "p2")] = p2

        def scan2_mul1(b, g):
            t0 = 4 * g
            p2 = live.pop((b, g, "p2"))
            y2 = ypool.tile([128, 516], f16, tag="y", name="y2")
            nc.vector.tensor_tensor_scan(
                y2[:, 1:513], dm_t[:], p2[:],
                initial=0.0, op0=MULT, op1=ADD,
            )
            m1 = mpool.tile([128, 512], f16, tag="m", name="m1")
            m1m = _TUNE["mul1_dve_mod"]
            eng = (nc.vector if m1m and (b * NG + g) % m1m == m1m - 1
                   else nc.gpsimd)
            eng.tensor_mul(
                m1[:],
                bpv[b][:, 0, t0 : t0 + 4, :].rearrange("p t h -> p (t h)"),
                y2[:, 0:512],
            )
            live[(b, g, "m1")] = m1

        def scan1_fmms(b, g, mm_final):
            m1 = live.pop((b, g, "m1"))
            if not mm_final:
                n1 = n1pool.tile([128, 512], f16, tag="n1", name="n1")
                nc.vector.tensor_tensor_scan(
                    n1[:], dm_t[:], m1[:],
                    initial=0.0, op0=MULT, op1=ADD,
                )
                pf = pspool.tile([128, 512], f32, tag="ps", name="pf")
                # operand-swapped final matmul:
                # out[w', (t,h)] = sum_w L[w',w] n1[w,(t,h)]
                nc.tensor.matmul(
                    pf[:], lhsT=fr_t[:], rhs=n1[:], skip_group_check=True,
                )
                live[(b, g, "pf")] = pf
                return
            # double-MM final: w-cumsum + flip via data-lhsT matmuls, then
            # evacuate; the h-cumsum happens next tick as a second matmul
            pfa = pspool.tile([128, 512], f32, tag="ps", name="pfa")
            for tl in range(4):
                nc.tensor.matmul(
                    pfa[:, tl * 128 : (tl + 1) * 128],
                    lhsT=m1[:, tl * 128 : (tl + 1) * 128],
                    rhs=fr_t[:],
                    skip_group_check=True,
                )
            qa = n1pool.tile([128, 512], f16, tag="n1", name="qa")
            qm = _TUNE["qa_dve_mod"]
            if qm and (b * NG + g) % qm == 0:
                nc.vector.tensor_copy(qa[:], pfa[:])
            else:
                nc.scalar.copy(qa[:], pfa[:])
            live[(b, g, "qa")] = qa

        def fmms2(b, g):
            # h-cumsum via partition contraction: out[h',(t,w')] =
            # sum_h L[h',h] qa[h,(t,w')]
            if (b, g, "qa") not in live:
                return
            qa = live.pop((b, g, "qa"))
            pf = pspool.tile([128, 512], f32, tag="ps", name="pfb")
            nc.tensor.matmul(
                pf[:], lhsT=fr_t[:], rhs=qa[:], skip_group_check=True,
            )
            live[(b, g, "pf")] = pf

        def evac_dma(b, g):
            pf = live.pop((b, g, "pf"))
            half = g // 2
            if g % 2 == 0:
                live[(b, half, "st")] = stpool.tile(
                    [128, 1024], f16, tag="st", name="stg"
                )
            stg = live[(b, half, "st")]
            sm = _TUNE["stg_dve_mod"]
            if sm and (b * NG + g) % sm == sm - 1:
                nc.vector.tensor_copy(
                    stg[:, (g % 2) * 512 : (g % 2 + 1) * 512], pf[:],
                )
            else:
                nc.scalar.copy(
                    stg[:, (g % 2) * 512 : (g % 2 + 1) * 512], pf[:],
                )
            if b * NG + g >= _TUNE["split_dma_from"]:
                # tail units: DMA each 4-t group as soon as it is staged so
                # the final transfer is half-size and starts a tick earlier
                dst = outT[b, :, g * 4 : g * 4 + 4, :]
                nc.sync.dma_start(
                    dst,
                    stg[:, (g % 2) * 512 : (g % 2 + 1) * 512].rearrange(
                        "p (t w) -> p t w", t=4
                    ),
                )
                if g % 2 == 1:
                    del live[(b, half, "st")]
            elif g % 2 == 1:
                del live[(b, half, "st")]
                dst = outT[b, :, half * 8 : half * 8 + 8, :]
                nc.sync.dma_start(
                    dst,
                    stg[:].rearrange("p (t w) -> p t w", t=8),
                )

        # ---- schedule ----
        for b in range(BPC):
            make_bp(b)

        if _TUNE["warm_mms"]:
            warm = pspool.tile([128, 96], f32, tag="ps", name="warm")
            for _ in range(_TUNE["warm_mms"]):
                nc.tensor.matmul(
                    warm[0:96, :], lhsT=alpha_t[:], rhs=alpha_t[:],
                    skip_group_check=True,
                )

        # head: b0's einsum completely (its x halves stream in first),
        # optionally plus some of b1's pieces
        for i, piece in enumerate(einsum_pieces(0)):
            piece()
            if i == 1:
                zero_shift_cols(0)
        extras = []
        for b in range(1, BPC):
            extras.extend(einsum_pieces(b))
        extras_done_b = {1: 27, 2: 55, 3: 83}   # piece index finishing b's Bp
        ei = 0

        def run_extra():
            nonlocal ei
            if ei < len(extras):
                extras[ei]()
                for b, last in extras_done_b.items():
                    if ei == last:
                        zero_shift_cols(b)
                ei += 1

        for _ in range(_TUNE["b1_head"]):
            run_extra()

        # software-pipelined stage ticks over units (b, g), with the
        # remaining einsum pieces paced so each b's Bp completes just
        # before its stage units begin.  Every DVE op's producer is
        # issued at least one tick earlier, so the in-order DVE queue
        # never stalls on same-tick work.
        units = [(b, g) for b in range(BPC) for g in range(NG)]
        NU = len(units)

        # prefetch pair-1 x chunks a few ticks before b2's pieces need them
        x_prefetch = {4: [0], 5: [1], 6: [2, 3]}

        for u in range(NU + 5):
            for hb in x_prefetch.get(u, ()):
                ensure_x(2, hb)
            if u < NU:
                s3mms(*units[u])
            if 0 <= u - 1 < NU:
                scan3_mul2(*units[u - 1])
            if 0 <= u - 2 < NU:
                s2mms(*units[u - 2])
            if 0 <= u - 3 < NU:
                scan2_mul1(*units[u - 3])
            if 0 <= u - 4 < NU:
                scan1_fmms(
                    *units[u - 4],
                    _TUNE["fin_mm_from"] <= u - 4 < _TUNE["fin_mm_to"],
                )
            if 0 <= u - 5 < NU:
                fmms2(*units[u - 5])
                evac_dma(*units[u - 5])
            if u >= _TUNE["extra_start"]:
                for _ in range(_TUNE["extra_rate"]):
                    run_extra()

    nc.compile()
    return nc


def _build_module_general():
    """Fallback: arbitrary per-t discounts (original variant)."""
    import concourse.bass as bass
    import concourse.mybir as mybir
    import concourse.tile as tile
    from concourse import bacc
    from contextlib import ExitStack

    dt = mybir.dt
    f32, f16 = dt.float32, dt.float16

    nc = bacc.Bacc(
        "TRN2", target_bir_lowering=False, debug=False, num_devices=NCORES
    )
    xs = nc.declare_dram_parameter("xs", [BPC, C, H, W], f32, isOutput=False)
    alphaT = nc.declare_dram_parameter("alphaT", [128, KA * T], f16, isOutput=False)
    stat3T = nc.declare_dram_parameter("stat3T", [128, T * 128], f16, isOutput=False)
    frhs = nc.declare_dram_parameter("frhs", [128, T * 128], f16, isOutput=False)
    dmask = nc.declare_dram_parameter("dmask", [128, T * 128], f32, isOutput=False)
    out = nc.declare_dram_parameter("out", [BPC, T, H, W], f32, isOutput=True)

    HB = 32
    NHB = H // HB
    NG = T // 4
    MULT = mybir.AluOpType.mult
    ADD = mybir.AluOpType.add
    COPY = mybir.ActivationFunctionType.Copy

    with tile.TileContext(nc) as tc, ExitStack() as ctx:
        const_pool = ctx.enter_context(tc.tile_pool(name="const", bufs=1))
        xpool = ctx.enter_context(tc.tile_pool(name="xp", bufs=4))
        bppool = ctx.enter_context(tc.tile_pool(name="bp", bufs=4))
        ypool = ctx.enter_context(tc.tile_pool(name="yp", bufs=_TUNE["yp_bufs"]))
        mpool = ctx.enter_context(tc.tile_pool(name="mp", bufs=_TUNE["mp_bufs"]))
        n1pool = ctx.enter_context(tc.tile_pool(name="n1", bufs=_TUNE["n1_bufs"]))
        stpool = ctx.enter_context(tc.tile_pool(name="st", bufs=_TUNE["st_bufs"]))
        pspool = ctx.enter_context(tc.tile_pool(name="ps", bufs=8, space="PSUM"))

        alpha_t = const_pool.tile([128, KA * T], f16, tag="alpha")
        nc.sync.dma_start(alpha_t[:], alphaT[:])
        s3_t = const_pool.tile([128, T * 128], f16, tag="s3m")
        fr_t = const_pool.tile([128, T * 128], f16, tag="frm")
        dm_t = const_pool.tile([128, T * 128], f32, tag="dmm")

        def load_stage_consts_head():
            nc.sync.dma_start(s3_t[:, 0:512], stat3T[:, 0:512])
            nc.sync.dma_start(dm_t[:, 0:512], dmask[:, 0:512])
            nc.sync.dma_start(fr_t[:, 0:512], frhs[:, 0:512])

        def load_stage_consts_rest():
            nc.sync.dma_start(s3_t[:, 512:], stat3T[:, 512:])
            nc.sync.dma_start(dm_t[:, 512:], dmask[:, 512:])
            nc.sync.dma_start(fr_t[:, 512:], frhs[:, 512:])

        bp_tiles = {}
        bpv = {}

        def make_bp(pair):
            bp_tiles[pair] = [
                bppool.tile(
                    [128, KA * T * 128], f16, tag="bp", name=f"bp{pair}_{i}"
                )
                for i in range(2)
            ]
            bpv[pair] = [
                t[:].rearrange("p (k t h) -> p k t h", k=KA, t=T)
                for t in bp_tiles[pair]
            ]

        def einsum_units(pair):
            for hb in range(NHB):
                holder = {}

                def ensure_x(pair=pair, hb=hb, holder=holder):
                    if "xt" not in holder:
                        xt = xpool.tile([128, HB * W], f16, tag="x", name="xt")
                        src = xs[
                            2 * pair : 2 * pair + 2, :, hb * HB : (hb + 1) * HB, :
                        ]
                        nc.gpsimd.dma_start(
                            xt[:], src.rearrange("b c h w -> (b c) (h w)")
                        )
                        holder["xt"] = xt
                    return holder["xt"]

                joff = 0
                for ui, ng in enumerate((5, 5, 5, 5, 4, 4, 4)):
                    def unit(pair=pair, hb=hb, ng=ng, joff=joff,
                             ensure_x=ensure_x, ui=ui):
                        xt = ensure_x()
                        pts = [
                            pspool.tile(
                                [128, ng * KA * T], f32, tag="ps", name=f"pe{i}"
                            )
                            for i in range(2)
                        ]
                        for j in range(ng):
                            for par in range(2):
                                nc.tensor.matmul(
                                    pts[par][:, j * 96 : (j + 1) * 96],
                                    lhsT=xt[
                                        64 * par : 64 * par + 64,
                                        (joff + j) * W : (joff + j + 1) * W,
                                    ],
                                    rhs=alpha_t[64 * par : 64 * par + 64, :],
                                    tile_position=(64 * par, 0),
                                    skip_group_check=True,
                                )
                        h0 = hb * HB + joff
                        for par in range(2):
                            src_ap = pts[par][:].rearrange(
                                "p (j k t) -> p j k t", j=ng, k=KA
                            )
                            dst_ap = (
                                bp_tiles[pair][par][:]
                                .rearrange("p (k t h) -> p k t h", k=KA, t=T)[
                                    :, :, :, h0 : h0 + ng
                                ]
                                .rearrange("p k t j -> p j k t")
                            )
                            gi = hb * 7 + ui
                            if pair == 0 and par == 0 and gi % 3 != 2:
                                nc.vector.tensor_copy(dst_ap, src_ap)
                            else:
                                nc.scalar.copy(dst_ap, src_ap)
                    yield unit
                    joff += ng

        live = {}

        def s3mms(pair, par, g):
            v = bpv[pair][par]
            t0 = 4 * g
            p3 = pspool.tile([128, 512], f32, tag="ps", name="p3")
            for tl in range(4):
                t = t0 + tl
                nc.tensor.matmul(
                    p3[:, tl * 128 : (tl + 1) * 128],
                    lhsT=s3_t[:, t * 128 : (t + 1) * 128],
                    rhs=v[:, 2, t, :],
                    skip_group_check=True,
                )
            live[(pair, par, g, "p3")] = p3

        def scan3_mul2(pair, par, g):
            v = bpv[pair][par]
            t0 = 4 * g
            p3 = live.pop((pair, par, g, "p3"))
            y3 = ypool.tile([128, 516], f16, tag="y", name="y3")
            nc.vector.tensor_tensor_scan(
                y3[:, 1:513], dm_t[:, t0 * 128 : t0 * 128 + 512], p3[:],
                initial=0.0, op0=MULT, op1=ADD,
            )
            m2 = mpool.tile([128, 512], f16, tag="m", name="m2")
            nc.gpsimd.tensor_mul(
                m2[:].rearrange("p (t h) -> p t h", t=4),
                v[:, 1, t0 : t0 + 4, :],
                y3[:, 0:512].rearrange("p (t h) -> p t h", t=4),
            )
            nc.gpsimd.memset(
                m2[:].rearrange("p (t h) -> p t h", t=4)[:, :, 0:1], 0.0
            )
            live[(pair, par, g, "m2")] = m2

        def s2mms_scan2_mul1(pair, par, g):
            v = bpv[pair][par]
            t0 = 4 * g
            m2 = live.pop((pair, par, g, "m2"))
            p2 = pspool.tile([128, 512], f32, tag="ps", name="p2")
            for tl in range(4):
                t = t0 + tl
                nc.tensor.matmul(
                    p2[:, tl * 128 : (tl + 1) * 128],
                    lhsT=s3_t[:, t * 128 : (t + 1) * 128],
                    rhs=m2[:, tl * 128 : (tl + 1) * 128],
                    skip_group_check=True,
                )
            y2 = ypool.tile([128, 516], f16, tag="y", name="y2")
            nc.vector.tensor_tensor_scan(
                y2[:, 1:513], dm_t[:, t0 * 128 : t0 * 128 + 512], p2[:],
                initial=0.0, op0=MULT, op1=ADD,
            )
            m1 = mpool.tile([128, 512], f16, tag="m", name="m1")
            eng = nc.gpsimd if g % 2 == 1 else nc.vector
            eng.tensor_mul(
                m1[:].rearrange("p (t h) -> p t h", t=4),
                v[:, 0, t0 : t0 + 4, :],
                y2[:, 0:512].rearrange("p (t h) -> p t h", t=4),
            )
            eng.memset(
                m1[:].rearrange("p (t h) -> p t h", t=4)[:, :, 0:1], 0.0
            )
            live[(pair, par, g, "m1")] = m1

        def scan1_fmms_evac(pair, par, g):
            b = 2 * pair + par
            t0 = 4 * g
            m1 = live.pop((pair, par, g, "m1"))
            n1 = n1pool.tile([128, 512], f16, tag="n1", name="n1")
            nc.vector.tensor_tensor_scan(
                n1[:], dm_t[:, t0 * 128 : t0 * 128 + 512], m1[:],
                initial=0.0, op0=MULT, op1=ADD,
            )
            pf = pspool.tile([128, 512], f32, tag="ps", name="pf")
            for tl in range(4):
                t = t0 + tl
                nc.tensor.matmul(
                    pf[:, tl * 128 : (tl + 1) * 128],
                    lhsT=n1[:, tl * 128 : (tl + 1) * 128],
                    rhs=fr_t[:, t * 128 : (t + 1) * 128],
                    skip_group_check=True,
                )
            half = g // 2
            if g % 2 == 0:
                live[(pair, par, half, "st")] = stpool.tile(
                    [128, 8 * 128], f32, tag="st", name="stg"
                )
            stg = live[(pair, par, half, "st")]
            nc.scalar.activation(
                stg[:, (g % 2) * 512 : (g % 2 + 1) * 512],
                pf[:], COPY, scale=UNSCALE,
            )
            if g % 2 == 1:
                del live[(pair, par, half, "st")]
                dst = out[b, half * 8 : half * 8 + 8, :, :]
                nc.sync.dma_start(
                    dst.rearrange("t h w -> h t w"),
                    stg[:].rearrange("p (t w) -> p t w", t=8),
                )

        def stage_ticks(pair, extra=None):
            for k in range(NG + 3):
                for par in range(2):
                    if k < NG:
                        s3mms(pair, par, k)
                    if 0 <= k - 1 < NG:
                        scan3_mul2(pair, par, k - 1)
                    if 0 <= k - 2 < NG:
                        s2mms_scan2_mul1(pair, par, k - 2)
                    if 0 <= k - 3 < NG:
                        scan1_fmms_evac(pair, par, k - 3)
                if extra is not None:
                    for _ in range(3):
                        u = next(extra, None)
                        if u is not None:
                            u()

        make_bp(0)
        units0 = list(einsum_units(0))
        units0[0]()
        load_stage_consts_head()
        for u in units0[1:]:
            u()
        load_stage_consts_rest()
        make_bp(1)
        it1 = iter(list(einsum_units(1)))
        stage_ticks(0, extra=it1)
        for u in it1:
            u()
        stage_ticks(1)

    nc.compile()
    return nc


def _host_prep_uniform(alpha_1, alpha_2, alpha_3, d):
    a1scaled = alpha_1.T * (C1 * d * d)
    alphaT = np.concatenate(
        [a1scaled, alpha_2.T * C2, alpha_3.T * C3], axis=1
    ).astype(np.float16)
    alphaT_dup = np.concatenate([alphaT, alphaT], axis=0)  # [128, 96]

    idx = np.arange(H)
    E = idx[:, None] - idx[None, :]
    P = d ** np.maximum(E, 0)
    L = np.where(E >= 0, P, 0.0)
    Ls = np.where(E >= 1, P, 0.0)
    statT = np.ascontiguousarray(Ls.T).astype(np.float16)
    frhsT = np.ascontiguousarray(L.T).astype(np.float16)
    dmask = np.full((128, 512), np.float32(d), np.float32)
    dmask[:, 0::128] = 0.0
    return alphaT_dup, statT, frhsT, dmask


def _host_prep_general(alpha_1, alpha_2, alpha_3, ds):
    a1scaled = alpha_1.T * (C1 * ds[None, :] ** 2)
    alphaT = np.concatenate(
        [a1scaled, alpha_2.T * C2, alpha_3.T * C3], axis=1
    ).astype(np.float16)
    alphaT_dup = np.concatenate([alphaT, alphaT], axis=0)  # [128, 96]

    idx = np.arange(H)
    E = idx[:, None] - idx[None, :]
    stat3T = np.zeros((128, T * 128), np.float16)
    frhs = np.zeros((128, T * 128), np.float16)
    dmask = np.zeros((128, T * 128), np.float32)
    for t in range(T):
        d = ds[t]
        P = d ** np.maximum(E, 0)
        L = np.where(E >= 0, P, 0.0)
        Ls = np.where(E >= 1, P, 0.0)
        sl = slice(t * 128, (t + 1) * 128)
        stat3T[:, sl] = Ls.T.astype(np.float16)
        frhs[:, sl] = L.T.astype(np.float16)
        dmask[:, sl] = np.float32(d)
        dmask[:, t * 128] = 0.0
    return alphaT_dup, stat3T, frhs, dmask


def kernel(x, alpha_1, alpha_2, alpha_3, discount):
    global _LAST_KEY
    from concourse.bass_utils import run_bass_kernel_spmd

    x = np.ascontiguousarray(np.asarray(x, dtype=np.float32))
    a1 = np.asarray(alpha_1, np.float32)
    a2 = np.asarray(alpha_2, np.float32)
    a3 = np.asarray(alpha_3, np.float32)
    ds = np.asarray(discount, dtype=np.float64).reshape(T)
    uniform = bool(np.all(ds == ds[0]))

    if uniform:
        key = ("nc", "uniform")
        if key not in _CACHE:
            _CACHE[key] = _build_module_uniform()
        nc = _CACHE[key]
        _LAST_KEY = key
        alphaT_dup, statT, frhsT, dmask = _host_prep_uniform(
            a1, a2, a3, float(ds[0])
        )
        x16 = x.astype(np.float16)
        shared = {
            "alphaT": alphaT_dup,
            "statT": statT,
            "frhsT": frhsT,
            "dmask": dmask,
        }
        in_maps = [
            {"xs": x16[i * BPC : (i + 1) * BPC], **shared}
            for i in range(NCORES)
        ]
        res = run_bass_kernel_spmd(nc, in_maps, core_ids=list(range(NCORES)))
        outs = [res.results[i]["outT"] for i in range(NCORES)]
        full = np.concatenate(outs, axis=0).astype(np.float32)
        # per-(core-local b, t-half) layout: scan-final units wrote
        # [w, t, h], double-MM-final units wrote [h, t, w]
        K1, K2 = _TUNE["fin_mm_from"], _TUNE["fin_mm_to"]
        out = np.empty((B, T, H, W), np.float32)
        for lb in range(BPC):
            for half in range(T // 8):
                g0 = 2 * half                      # first g of this half
                flipped = K1 <= (lb * NG_HOST + g0) < K2  # (b, g)-major units
                # cores are concatenated on axis 0: global b = core*BPC + lb
                idxs = [c * BPC + lb for c in range(NCORES)]
                blk = full[idxs][:, :, half * 8 : half * 8 + 8, :]
                if flipped:
                    # stored [h, t, w] -> want [t, h, w]
                    out[idxs, half * 8 : half * 8 + 8] = blk.transpose(
                        0, 2, 1, 3
                    )
                else:
                    # stored [w, t, h] -> want [t, h, w]
                    out[idxs, half * 8 : half * 8 + 8] = blk.transpose(
                        0, 2, 3, 1
                    )
        return out * np.float32(UNSCALE)

    key = ("nc", "general")
    if key not in _CACHE:
        _CACHE[key] = _build_module_general()
    nc = _CACHE[key]
    _LAST_KEY = key
    alphaT_dup, stat3T, frhs, dmask = _host_prep_general(a1, a2, a3, ds)
    shared = {
        "alphaT": alphaT_dup,
        "stat3T": stat3T,
        "frhs": frhs,
        "dmask": dmask,
    }
    in_maps = [
        {"xs": x[i * BPC : (i + 1) * BPC], **shared} for i in range(NCORES)
    ]
    res = run_bass_kernel_spmd(nc, in_maps, core_ids=list(range(NCORES)))
    outs = [res.results[i]["out"] for i in range(NCORES)]
    return np.concatenate(outs, axis=0).astype(np.float32)


if __name__ == "__main__":
    import reference as ref

    inputs = {k: np.asarray(v) for k, v in ref.setup_inputs().items()}
    got = kernel(**inputs)
    print("kernel output shape:", got.shape, got.dtype)
